# revision 83
# baseline (speedup 1.0000x reference)
"""Trainium2 Bass kernel for nn_CoLL_78065325572576 (moe_routing).

Reference computation (per voxel v of x[B,H,W,C], nb=8 bins):
    b_v   = floor(8*x_v)                       (bin index)
    temp  = co[i, b_v] * x_v                   (8 channels)
    conv  = depthwise 3x3x3 conv over (H,W,C)  (SAME pad, 8 channels)
    out_v = conv[v, b_v] + bias[b_v]

Kernel formulation (all equalities exact):
    s_q[v]  = x_v * 1[b_v == q]                 (mask-routed fields)
    out_v   = sum_p 1[b_v==p] * ( sum_{dv,q} K[dv,p]*co[p,q] * s_q[v+dv] + bias[p] )

Device mapping (per core, pure data-parallel over 8 cores = batch x W-half):
  - per h-window (16 rows, stride 14): ONE partition-replicating DMA loads
    x rows into all 8 bin-groups (src AP [[0,8],[row,16],[1,F]]); window
    processing is split into quarters so no single DVE op blocks the
    select chain for long, and runs two windows ahead of the conv.
  - ROUTE (custom DVE op): s = x masked per bin-group, bf16.
  - SPLIT: ScalarE casts s -> s_hi (fp8 e4m3); GPSIMD(Pool) computes
    s_lo = s - s_hi (fp8) into the second plane of the same tile.
  - CONV: fp8 DoubleRow matmuls on TensorE (0.5 cycles/row = 2x bf16):
    the product w*s is computed as w_hi*s_hi + w_hi*s_lo + w_lo*s_hi
    (the dropped w_lo*s_lo term is ~2^-8 relative; additionally the
    w_lo slots of the NDROP least-significant taps are dropped, chosen
    per-input by a variance proxy).  Slots pack into 12 DoubleRow
    matmuls per (window, 8w) chunk:
      * 9 cross-plane pairs  (w_hi x s_hi, w_hi x s_lo), one per tap
      * 3 same-plane pairs   (w_lo x s_hi, kept taps two at a time)
    The routed planes use an UNPADDED 64-pitch c-layout so each DR slot
    is one contiguous 512-element span (the DR datapath needs 2-D slot
    views); ~9 tiny DoubleRow "correction" matmuls (free=8, strided rhs)
    accumulate the negated c-border wrap-around contamination into a
    per-window PSUM tile pcw[128,2,2,8], drained to SBUF by ScalarE once
    per window; a strided DVE add folds it into the 16 border columns of
    ps -- exactly equivalent to SAME zero padding of the C dim.
  - SELECT (custom DVE op): masked = (x in bin p) ? g + bias[p] : 0 (bf16).
  - REDUCE: per-window banded bf16 matmul scatters the 14 valid rows into
    two single-bank PSUM accumulators [128, 512] (one per 8w half, 10
    windows accumulate; bufs=1 each frees banks for 5-deep ps pipelining).
  - Two ScalarE half-drains + output DMAs per stripe (overlap the tail).
"""

import numpy as np
import ml_dtypes

NB = 8
B, H, W, C = 4, 128, 128, 64
WS = 64            # output W per core
WH = WS + 2        # input W incl. halo
WIN = 16           # h-window rows (one partition group)
VALID = 14         # valid output rows per window
NWIN = 10          # h windows (stride 14): covers h in [0,128)
NCORES = 8
NSTRIPES = 4
WSTR = WS // NSTRIPES        # 16 output w per stripe
WSTR_IN = WSTR + 2           # 18 input w per stripe
FSTR = WSTR_IN * C           # 1152 (18w x 64c, no pad)
PAD = 4                      # head/tail pad per window block (slot over/underrun)
WBLK = PAD + FSTR + PAD      # 1160: padded window block in sl8
WPAIR = 2 * WBLK             # 2320: window block (hi+lo planes) in sl8
TAPS = [(dw, dc) for dc in (0, -1, 1) for dw in (-1, 0, 1)]  # dc=0 first
CTAPS = [3, 4, 5, 6, 7, 8]   # taps with dci != 0 (need c-border correction)
NDROP = 3                    # w_lo slots dropped (9-NDROP must be even)
_DROP_TAPS = [(0, 1, 2)]     # taps whose w_lo is dropped (set by consts)
PP = WH * C                  # x_s row pitch (elements)
PH = 14 * (NWIN - 1) + WIN   # padded shard height: all windows in-bounds (142)

_prog_cache: dict = {}


# --------------------------------------------------------------------------- #
# custom DVE ops (registered at import into concourse.dve_ops)                #
# --------------------------------------------------------------------------- #

def _register_ops():
    from concourse import dve_ops
    from concourse.dve_spec import (
        Spec, Src0, Src1, C0, C1, C2, Zero, lower, select, _has_src1,
    )
    from concourse.dve_uop import DveOpSpec

    if "ANT_ROUTE_BIN8" in dve_ops._SUB_OPCODE_FOR_NAME:
        ops = {op.name: op for op in dve_ops.OPS}
        return ops["ANT_ROUTE_BIN8"], ops["ANT_SEL_BIN8"]

    def reg(name, spec, subdim=False):
        row = dve_ops._CUSTOM_DVE_ROW_BASE + len(dve_ops.OPS)
        assert row < 0x20, "custom DVE opcode rows exhausted"
        dve_ops._SUB_OPCODE_FOR_NAME[name] = row
        shas = {}
        for ver in ("v3", "v4"):
            try:
                s = DveOpSpec(name=name, opcode=row,
                              uops=lower(spec, ver=ver),
                              rd1_en=_has_src1(spec))
                shas[ver] = s.sha(ver)
            except Exception:
                pass
        op = dve_ops.DveOp(name, spec, subdim=subdim, uops_sha=shas)
        dve_ops.OPS.append(op)
        dve_ops.CUSTOM_DVE_SPECS[name] = spec
        return op

    def _bc(v):
        return v if isinstance(v, float) else np.asarray(v).reshape(-1, 1)

    # s = x if (x >= lo) & (x < hi) else 0   (lo/hi per-partition scalars)
    route = reg("ANT_ROUTE_BIN8", Spec(
        body=select((Src0 >= C0) & (Src0 < C1), Src0, Zero),
        reference=lambda in0, in1, s0, s1, imm2: np.where(
            (in0 >= _bc(s0)) & (in0 < _bc(s1)), in0, 0.0).astype(np.float32),
    ))

    # masked = (x >= lo) & (x < lo + width) ? g + bias : 0
    #   in0 = g (PSUM), in1 = x (center voxel), s0 = lo, s1 = bias,
    #   imm2 = 1/8 bin width (compile-time literal)
    selb = reg("ANT_SEL_BIN8", Spec(
        body=select((Src1 >= C0) & (Src1 < (C0 + C2)), Src0 + C1, Zero),
        reference=lambda in0, in1, s0, s1, imm2: np.where(
            (in1 >= _bc(s0)) & (in1 < (_bc(s0) + imm2)),
            in0 + _bc(s1), 0.0).astype(np.float32),
    ))
    return route, selb


# --------------------------------------------------------------------------- #
# host-side constant construction                                             #
# --------------------------------------------------------------------------- #

def _corr_lo_pairs(drops):
    """Pairs of taps for the w_lo c-border correction DRs, grouped per side
    (both slots of a DR accumulate into the same pc region)."""
    pairs = []
    for side, lst in ((0, [t for t in (3, 4, 5) if t not in drops]),
                      (1, [t for t in (6, 7, 8) if t not in drops])):
        for k in range(0, len(lst), 2):
            pairs.append((side, lst[k], lst[k + 1] if k + 1 < len(lst)
                          else None))
    return pairs


def _band_lhsT(dw_kernel, co_matrix, dwi, dci):
    """lhsT[(q,hs),(p,hs')] = K[dh+1, dwi+1, dci+1, p] * co[p,q], dh=hs-hs',
    for hs' in [1,15), |dh| <= 1."""
    K = np.asarray(dw_kernel, np.float32)       # [3,3,3,1,8]
    co = np.asarray(co_matrix, np.float32)      # [8,8]
    lhsT = np.zeros((128, 128), np.float32)
    hsp = np.arange(1, 15)
    for q in range(NB):
        for p in range(NB):
            for dh in (-1, 0, 1):
                a = K[dh + 1, dwi + 1, dci + 1, 0, p] * co[p, q]
                lhsT[q * WIN + hsp + dh, p * WIN + hsp] = a
    return lhsT


def _make_consts(co_matrix, dw_kernel, dw_bias):
    E4 = ml_dtypes.float8_e4m3
    bands = [_band_lhsT(dw_kernel, co_matrix, dwi, dci)
             for (dwi, dci) in TAPS]                       # 9 x [128,128] f32
    hi = [b.astype(E4).astype(np.float32) for b in bands]  # w_hi
    lo = [b - h for b, h in zip(bands, hi)]                # w_lo (exact resid)

    # drop the w_lo slots of the NDROP least-significant taps so the
    # remaining slots pack into (9-NDROP)/2 DoubleRow matmuls (program
    # structure depends on this choice; _build_program reads _DROP_TAPS).
    # Significance proxy: per-hypothesis std of the dropped term,
    # sum_{dh,q} w_lo^2 E[s_q^2] with E[s_q^2] = E[x^2 1(bin q)], x~U[0,1).
    K5 = np.asarray(dw_kernel, np.float32)
    co5 = np.asarray(co_matrix, np.float32)
    m2 = np.array([((q + 1) ** 3 - q ** 3) / (3.0 * 8 ** 3) for q in range(8)])
    prox = []
    for (dwi, dci) in TAPS:
        v = 0.0
        for dh in range(3):
            wt = K5[dh, dwi + 1, dci + 1, 0, :][:, None] * co5
            wlo = wt - wt.astype(E4).astype(np.float32)
            v = v + (wlo ** 2 * m2[None, :]).sum(axis=1)
        prox.append(np.sqrt(v).max())
    drops = tuple(sorted(int(t) for t in np.argsort(prox)[:NDROP]))
    _DROP_TAPS[0] = drops
    lo_taps = [t for t in range(9) if t not in drops]
    npair = len(lo_taps) // 2

    # conv DoubleRow weights: [128, 9+npair, 2, 128]
    cw8 = np.zeros((128, 9 + npair, 2, 128), np.float32)
    for t in range(9):
        cw8[:, t, 0, :] = hi[t]        # slot0: w_hi x s_hi
        cw8[:, t, 1, :] = hi[t]        # slot1: w_hi x s_lo
    for j in range(npair):
        t0, t1 = lo_taps[2 * j], lo_taps[2 * j + 1]
        cw8[:, 9 + j, 0, :] = lo[t0]   # slot0: w_lo[t0] x s_hi
        cw8[:, 9 + j, 1, :] = lo[t1]

    # c-border correction weights (subtract the wrap-around contamination):
    # j=0..5: slot0 hits the s_hi plane, slot1 the s_lo plane, both -w_hi
    #         (cancels w_hi*cont_hi + w_hi*cont_lo for each dci!=0 tap).
    # j=6..9: -w_lo slots, both on the s_hi plane, taps paired two at a
    #         time per side (cancels w_lo*cont_hi).
    # w_lo contamination is only cancelled for taps whose main w_lo slot
    # exists (a dropped tap added no w_lo contribution to cancel).
    lo_pairs = _corr_lo_pairs(drops)
    cwc = np.zeros((128, 6 + len(lo_pairs), 2, 128), np.float32)
    for j, ti in enumerate(CTAPS):
        cwc[:, j, 0, :] = -hi[ti]
        cwc[:, j, 1, :] = -hi[ti]
    for i, (side, t0, t1) in enumerate(lo_pairs):
        cwc[:, 6 + i, 0, :] = -lo[t0]
        if t1 is not None:
            cwc[:, 6 + i, 1, :] = -lo[t1]

    # red_w[hw][p*16+hs, h] = 1 iff h == 14*hw + hs - 1, hs in [1,15)
    red_w = np.zeros((NWIN, 128, 128), np.float32)
    for hw in range(NWIN):
        for p in range(NB):
            for hs in range(1, 15):
                h = 14 * hw + hs - 1
                if 0 <= h < H:
                    red_w[hw, p * WIN + hs, h] = 1.0
    part = np.arange(128)
    bins_lo = ((part // WIN) / NB).astype(np.float32).reshape(128, 1)
    bias_p = np.asarray(dw_bias, np.float32)[part // WIN].reshape(128, 1)
    return {
        "cw8": cw8.astype(E4),
        "cwc": cwc.astype(E4),
        "red_w": red_w.astype(ml_dtypes.bfloat16),
        "bins_lo": bins_lo,
        "bias_p": bias_p,
    }


def _shard(x, core):
    """Per-core input: [PH, WH, C] with zero h-halo rows and w-halo cols."""
    b, wh = core // 2, core % 2
    xp = np.zeros((PH, WH, C), np.float32)
    lo, hi = wh * WS - 1, wh * WS + WS + 1
    slo, shi = max(0, lo), min(W, hi)
    xp[1:H + 1, slo - lo:shi - lo, :] = x[b, :, slo:shi, :]
    return xp


# --------------------------------------------------------------------------- #
# device program                                                              #
# --------------------------------------------------------------------------- #

def _build_program(reps=1):
    import concourse.mybir as mybir
    import concourse.tile as tile
    from concourse import bacc
    import bass_rust

    def raw_ap(base_ap, dims, offset):
        a = base_ap.copy()
        a.ap = bass_rust.VecI64Pair(dims)
        a.offset = offset
        return a

    def raw_free(base_ap, dims, extra_off):
        """Keep the partition dim of an SBUF slice, replace its free dims."""
        a = base_ap.copy()
        a.ap = bass_rust.VecI64Pair([list(base_ap.ap[0])] + dims)
        a.offset = base_ap.offset + extra_off
        return a

    ROUTE, SELB = _register_ops()
    f32 = mybir.dt.float32
    bf16 = mybir.dt.bfloat16
    e4 = mybir.dt.float8e4
    DR = mybir.MatmulPerfMode.DoubleRow

    drops = _DROP_TAPS[0]
    lo_taps = [t for t in range(9) if t not in drops]
    npair = len(lo_taps) // 2
    corr_lo = _corr_lo_pairs(drops)
    NCW = 9 + npair
    NCC = 6 + len(corr_lo)
    nc = bacc.Bacc("TRN2", target_bir_lowering=False, debug=False)
    x_d = nc.dram_tensor("x_s", [PH, WH, C], f32, kind="ExternalInput")
    cw_d = nc.dram_tensor("cw8", [128, NCW, 2, 128], e4, kind="ExternalInput")
    cc_d = nc.dram_tensor("cwc", [128, NCC, 2, 128], e4, kind="ExternalInput")
    rw_d = nc.dram_tensor("red_w", [NWIN, 128, 128], bf16,
                          kind="ExternalInput")
    lo_d = nc.dram_tensor("bins_lo", [128, 1], f32, kind="ExternalInput")
    bi_d = nc.dram_tensor("bias_p", [128, 1], f32, kind="ExternalInput")
    out_d = nc.dram_tensor("out_s", [H, WS, C], f32, kind="ExternalOutput")

    with tile.TileContext(nc) as tc:
        with (
            tc.tile_pool(name="const", bufs=1) as cpool,
            tc.tile_pool(name="xr", bufs=2) as xrpool,
            tc.tile_pool(name="s16", bufs=3) as s16pool,
            tc.tile_pool(name="sl8", bufs=2) as sl8pool,
            tc.tile_pool(name="mk", bufs=5) as mkpool,
            tc.tile_pool(name="ost", bufs=2) as ostpool,
            tc.tile_pool(name="ps", bufs=5, space="PSUM") as pspool,
            tc.tile_pool(name="ps2", bufs=1, space="PSUM") as ps2pool,
            tc.tile_pool(name="pc", bufs=1, space="PSUM") as pcpool,
        ):
            # startup critical chain: lo/hi gate the first ROUTE, cw8 gates the
            # first conv matmul; load those before the bulkier rw (only
            # needed once the first chunk's SELB completes).
            # lo/bi ride the gpsimd queue so the sync queue's head starts the
            # first xrm window immediately.
            lo = cpool.tile([128, 1], f32)
            nc.gpsimd.dma_start(lo[:], lo_d[:])
            bi = cpool.tile([128, 1], f32)
            nc.gpsimd.dma_start(bi[:], bi_d[:])
            hi = cpool.tile([128, 1], f32)
            nc.vector.tensor_scalar_add(hi[:], lo[:], 1.0 / NB)
            # big consts go on the Act queue: the sync queue must start the
            # first xrm windows immediately and the Pool queue must not delay
            # the first s_lo sub; Act is idle until the first s_hi convert.
            cw = cpool.tile([128, NCW, 2, 128], e4)
            nc.scalar.dma_start(cw[:], cw_d[:])
            cc = cpool.tile([128, NCC, 2, 128], e4)
            nc.scalar.dma_start(cc[:], cc_d[:])
            rw = cpool.tile([128, NWIN * 128], bf16)
            nc.scalar.dma_start(
                rw[:, :],
                raw_ap(rw_d[0], [[128, 128], [128 * 128, NWIN], [1, 128]], 0))

            # PE p-state warmup: ~16 throwaway matmuls on zeroed data keep
            # the PE busy through its 3us clock ramp while the first x
            # window streams in, so real convs start at full clock.  The
            # huge priority offset parks them below all real work in the
            # scheduler's ready heap.
            warm = cpool.tile([128, 512], bf16)
            nc.vector.memset(warm[:], 0.0)
            wps = pspool.tile([128, 512], f32, tag="ps", name="warm_ps")
            with tc.high_priority(offset=-(1 << 20)):
                for i in range(16):
                    nc.tensor.matmul(wps[:], warm[:, 0:128], warm[:],
                                     start=(i == 0), stop=(i == 15))

            for rep in range(reps):
              for st in range(NSTRIPES):
                  wb = st * WSTR
                  # ---- load x replicated into (q, hs) per window ----------- #
                  # padded row r = h+1: window hw needs h = 14*hw-1 .. 14*hw+14
                  # -> padded rows 14*hw .. 14*hw+15, all in-bounds.
                  xrm = xrpool.tile([128, NWIN, FSTR], f32, tag="xrm")

                  def xdma(hw, st=st, xrm=xrm, halves=False):
                      # halves=True: two half-window DMAs so the first route
                      # can start sooner (startup only).
                      parts = ((0, FSTR // 2), (FSTR // 2, FSTR)) if halves \
                          else ((0, FSTR),)
                      for f0, f1 in parts:
                          nc.sync.dma_start(
                              xrm[:, hw, f0:f1],
                              raw_ap(x_d[0:WIN, 0:WSTR_IN, :],
                                     [[0, 8], [PP, WIN], [1, f1 - f0]],
                                     14 * hw * PP + st * WSTR * C + f0))

                  def xwin(hw):
                      return xrm[:, hw, :]

                  # ---- route to bf16, split into fp8 hi/lo planes ---------- #
                  # window DMAs are emitted with prefetch depth 2 so the first
                  # route isn't stuck behind 10 queued DMA configs; window
                  # processing is interleaved with the conv chunks below so
                  # each engine's FIFO stays in pipeline order.
                  xdma(0, halves=(st == 0))
                  xdma(1, halves=(st == 0))
                  sl8 = sl8pool.tile([128, NWIN, 2, WBLK], e4, tag="sl8")
                  nc.gpsimd.memset(sl8[:, :, :, 0:PAD], 0.0)
                  nc.gpsimd.memset(sl8[:, :, :, WBLK - PAD:WBLK], 0.0)

                  def process_window(hw, st=st, sl8=sl8, halves=False):
                      if hw + 2 < NWIN:
                          xdma(hw + 2)
                      s16 = s16pool.tile([128, FSTR], bf16, tag="s16",
                                         name=f"s16_{st}_{hw}")
                      if halves:
                          # thirds: shorter DVE ops so a ready select op is
                          # blocked behind at most ~1/3 of a route.
                          parts = ((0, 288), (288, 576), (576, 864), (864, FSTR))
                      else:
                          parts = ((0, FSTR),)
                      for f0, f1 in parts:
                          nc.vector._custom_dve(
                              ROUTE, out=s16[:, f0:f1],
                              in0=xwin(hw)[:, f0:f1], s0=lo[:], s1=hi[:])
                          nc.scalar.copy(sl8[:, hw, 0, PAD + f0:PAD + f1],
                                         s16[:, f0:f1])
                          nc.gpsimd.tensor_sub(
                              sl8[:, hw, 1, PAD + f0:PAD + f1],
                              s16[:, f0:f1], sl8[:, hw, 0, PAD + f0:PAD + f1])

                  # two-window lookahead: the 3-engine route->s_hi->s_lo chain
                  # takes about one window's PE time, so it must run a full
                  # window ahead of the conv chunks.  The very first windows
                  # are processed in halves to shorten the cold-start chain.
                  process_window(0, halves=True)
                  process_window(1, halves=(st == 0))
                  sl8b = sl8[:, 0, 0, 0:512]   # AP base for raw free-dim APs

                  # ---- conv + select + stripe-accumulated reduce ----------- #
                  # two independent single-bank accumulators (one per wc
                  # half) with bufs=1: each is drained well before the next
                  # stripe's first reduce into it, so double-buffering them
                  # would waste PSUM banks better spent on ps depth.
                  p2h = (ps2pool.tile([128, 512], f32, tag="p2a",
                                      name=f"p2a_{st}"),
                         ps2pool.tile([128, 512], f32, tag="p2b",
                                      name=f"p2b_{st}"))
                  pend = []

                  def flush_tail(pend=pend, st=st, p2h=p2h):
                      if not pend:
                          return
                      ps, hw, wc, pcs = pend.pop(0)
                      # fold the border corrections into ps, then select.
                      # element order (w, side): ps col = 64*w + 63*side,
                      # pcs elem = wc*16 + 8*side + w.
                      ps_b = raw_free(ps[:, 0:8], [[C, 8], [C - 1, 2]], 0)
                      pc_b = raw_free(pcs[:, 0:8], [[1, 8], [8, 2]], wc * 16)
                      mk = mkpool.tile([128, 512], bf16, tag="mk",
                                       name=f"mk_{st}_{hw}_{wc}")
                      xcen = xwin(hw)[:, (wc * 8 + 1) * C:(wc * 8 + 9) * C]
                      # the fix->select->reduce chain gates PE; let it win
                      # ties against lookahead work in the scheduler heap.
                      with tc.high_priority(offset=150):
                          nc.vector.tensor_add(ps_b, ps_b, pc_b)
                          nc.vector._custom_dve(
                              SELB, out=mk[:], in0=ps[:],
                              in1=xcen, s0=lo[:], s1=bi[:], imm2=1.0 / NB)
                      nc.tensor.matmul(
                          p2h[wc][:, :],
                          rw[:, hw * 128:(hw + 1) * 128], mk[:],
                          start=(hw == 0), stop=(hw == NWIN - 1))

                  for hw in range(NWIN):
                      if hw + 2 < NWIN:
                          process_window(hw + 2)
                      blk = hw * WPAIR
                      # per-window correction accumulator [wc, side, 8] and
                      # its SBUF drain (one DVE copy per window, not chunk).
                      pcw = pcpool.tile([128, 2, 2, 8], f32, tag="pc",
                                        name=f"pc_{st}_{hw}")
                      pcs = mkpool.tile([128, 32], f32, tag="pcs",
                                        name=f"pcs_{st}_{hw}")
                      for wc in range(WSTR // 8):
                          w0 = wc * 8 + 1
                          ps = pspool.tile([128, 512], f32, tag="ps",
                                           name=f"ps_{st}_{hw}_{wc}")
                          nmm = 0
                          # 9 cross-plane pairs: w_hi*(s_hi, s_lo)
                          for t, (dwi, dci) in enumerate(TAPS):
                              F = blk + PAD + (w0 + dwi) * C + dci
                              rhs = raw_free(sl8b, [[WBLK, 2], [1, 512]], F)
                              nc.tensor.matmul(
                                  ps[:], cw[:, t, :, :], rhs,
                                  start=(nmm == 0), stop=False, perf_mode=DR)
                              nmm += 1
                          # same-plane pairs: w_lo * s_hi, taps two at a time
                          for j in range(npair):
                              t0, t1 = lo_taps[2 * j], lo_taps[2 * j + 1]
                              dw0, dc0 = TAPS[t0]
                              dw1, dc1 = TAPS[t1]
                              F0 = blk + PAD + (w0 + dw0) * C + dc0
                              delta = (dw1 - dw0) * C + (dc1 - dc0)
                              rhs = raw_free(sl8b, [[delta, 2], [1, 512]], F0)
                              nc.tensor.matmul(
                                  ps[:], cw[:, 9 + j, :, :], rhs,
                                  start=False, stop=(j == npair - 1),
                                  perf_mode=DR)
                              nmm += 1
                          # c-border corrections (free=8, strided rhs) into
                          # pcw[:, wc, side, :]; side 0 fixes out c=0 (dci=-1
                          # taps), side 1 fixes out c=63 (dci=+1 taps).

                          def fc_off(ti):
                              dwi, dci = TAPS[ti]
                              if dci == -1:
                                  return blk + PAD + (w0 + dwi - 1) * C + (C - 1)
                              return blk + PAD + (w0 + dwi + 1) * C

                          ncorr = 6 + len(corr_lo)
                          for j, ti in enumerate(CTAPS):
                              side = 0 if TAPS[ti][1] == -1 else 1
                              rhs = raw_free(sl8b, [[WBLK, 2], [C, 8]],
                                             fc_off(ti))
                              nc.tensor.matmul(
                                  pcw[:, wc, side, :], cc[:, j, :, :], rhs,
                                  start=(j == 0), stop=False, perf_mode=DR)
                          # -w_lo slots (hi plane, taps paired per side)
                          for i, (side, t0, t1) in enumerate(corr_lo):
                              F0 = fc_off(t0)
                              if t1 is not None:
                                  delta = fc_off(t1) - F0
                              else:
                                  delta = C if side == 0 else -C
                              rhs = raw_free(sl8b, [[delta, 2], [C, 8]], F0)
                              nc.tensor.matmul(
                                  pcw[:, wc, side, :], cc[:, 6 + i, :, :], rhs,
                                  start=False, stop=(6 + i == ncorr - 1),
                                  perf_mode=DR)
                          pend.append((ps, hw, wc, pcs))
                          if wc == 1:
                              # drain both chunks' corrections to SBUF (the
                              # DVE add may read only one PSUM operand).
                              with tc.high_priority(offset=150):
                                  nc.scalar.copy(pcs[:, :], pcw[:, :, :, :])
                          if len(pend) > 2:
                              flush_tail()
                  while pend:
                      flush_tail()

                  # ---- drain stripe + store (two halves so the wc=0 half
                  # overlaps the wc=1 tail) --------------------------------- #
                  ost = ostpool.tile([128, 1024], f32, tag="ost",
                                     name=f"ost_{st}")
                  for wc in range(2):
                      nc.scalar.copy(ost[:, wc * 512:(wc + 1) * 512],
                                     p2h[wc][:, :])
                      nc.sync.dma_start(
                          raw_ap(out_d[0:H, 0:WSTR, :],
                                 [[WS * C, 128], [1, 512]],
                                 wb * C + wc * 512),
                          ost[:, wc * 512:(wc + 1) * 512])

    nc.compile()
    return nc


def _get_program(reps=1):
    key = (reps, _DROP_TAPS[0])
    if key not in _prog_cache:
        _prog_cache[key] = _build_program(reps)
    return _prog_cache[key]


# --------------------------------------------------------------------------- #
# entry point                                                                 #
# --------------------------------------------------------------------------- #

def kernel(x, co_matrix, dw_kernel, dw_bias):
    from concourse.bass_utils import run_bass_kernel_spmd

    x = np.asarray(x, np.float32)
    consts = _make_consts(co_matrix, dw_kernel, dw_bias)
    nc = _get_program()

    in_maps = []
    for core in range(NCORES):
        m = {"x_s": _shard(x, core)}
        m.update(consts)
        in_maps.append(m)

    res = run_bass_kernel_spmd(nc, in_maps, core_ids=list(range(NCORES)))
    out = np.zeros((B, H, W, C), np.float32)
    for core in range(NCORES):
        b, wh = core // 2, core % 2
        out[b, :, wh * WS:(wh + 1) * WS, :] = res.results[core]["out_s"]
    return out


# revision 86
# speedup vs baseline: 1.0013x; 1.0013x over previous
"""Trainium2 Bass kernel for nn_CoLL_78065325572576 (moe_routing).

Reference computation (per voxel v of x[B,H,W,C], nb=8 bins):
    b_v   = floor(8*x_v)                       (bin index)
    temp  = co[i, b_v] * x_v                   (8 channels)
    conv  = depthwise 3x3x3 conv over (H,W,C)  (SAME pad, 8 channels)
    out_v = conv[v, b_v] + bias[b_v]

Kernel formulation (all equalities exact):
    s_q[v]  = x_v * 1[b_v == q]                 (mask-routed fields)
    out_v   = sum_p 1[b_v==p] * ( sum_{dv,q} K[dv,p]*co[p,q] * s_q[v+dv] + bias[p] )

Device mapping (per core, pure data-parallel over 8 cores = batch x W-half):
  - per h-window (16 rows, stride 14): ONE partition-replicating DMA loads
    x rows into all 8 bin-groups (src AP [[0,8],[row,16],[1,F]]); window
    processing is split into quarters so no single DVE op blocks the
    select chain for long, and runs two windows ahead of the conv.
  - ROUTE (custom DVE op): s = x masked per bin-group, bf16.
  - SPLIT: ScalarE casts s -> s_hi (fp8 e4m3); GPSIMD(Pool) computes
    s_lo = s - s_hi (fp8) into the second plane of the same tile.
  - CONV: fp8 DoubleRow matmuls on TensorE (0.5 cycles/row = 2x bf16):
    the product w*s is computed as w_hi*s_hi + w_hi*s_lo + w_lo*s_hi
    (the dropped w_lo*s_lo term is ~2^-8 relative; additionally the
    w_lo slots of the NDROP least-significant taps are dropped, chosen
    per-input by a variance proxy).  Slots pack into 12 DoubleRow
    matmuls per (window, 8w) chunk:
      * 9 cross-plane pairs  (w_hi x s_hi, w_hi x s_lo), one per tap
      * 3 same-plane pairs   (w_lo x s_hi, kept taps two at a time)
    The routed planes use an UNPADDED 64-pitch c-layout so each DR slot
    is one contiguous 512-element span (the DR datapath needs 2-D slot
    views); ~9 tiny DoubleRow "correction" matmuls (free=8, strided rhs)
    accumulate the negated c-border wrap-around contamination into a
    per-window PSUM tile pcw[128,2,2,8], drained to SBUF by ScalarE once
    per window; a strided DVE add folds it into the 16 border columns of
    ps -- exactly equivalent to SAME zero padding of the C dim.
  - SELECT (custom DVE op): masked = (x in bin p) ? g + bias[p] : 0 (bf16).
  - REDUCE: per-window banded bf16 matmul scatters the 14 valid rows into
    two single-bank PSUM accumulators [128, 512] (one per 8w half, 10
    windows accumulate; bufs=1 each frees banks for 5-deep ps pipelining).
  - Two ScalarE half-drains + output DMAs per stripe (overlap the tail).
"""

import numpy as np
import ml_dtypes

NB = 8
B, H, W, C = 4, 128, 128, 64
WS = 64            # output W per core
WH = WS + 2        # input W incl. halo
WIN = 16           # h-window rows (one partition group)
VALID = 14         # valid output rows per window
NWIN = 10          # h windows (stride 14): covers h in [0,128)
NCORES = 8
NSTRIPES = 4
WSTR = WS // NSTRIPES        # 16 output w per stripe
WSTR_IN = WSTR + 2           # 18 input w per stripe
FSTR = WSTR_IN * C           # 1152 (18w x 64c, no pad)
PAD = 4                      # head/tail pad per window block (slot over/underrun)
WBLK = PAD + FSTR + PAD      # 1160: padded window block in sl8
WPAIR = 2 * WBLK             # 2320: window block (hi+lo planes) in sl8
TAPS = [(dw, dc) for dc in (0, -1, 1) for dw in (-1, 0, 1)]  # dc=0 first
CTAPS = [3, 4, 5, 6, 7, 8]   # taps with dci != 0 (need c-border correction)
NDROP = 3                    # w_lo slots dropped (9-NDROP must be even)
_DROP_TAPS = [(0, 1, 2)]     # taps whose w_lo is dropped (set by consts)
PP = WH * C                  # x_s row pitch (elements)
PH = 14 * (NWIN - 1) + WIN   # padded shard height: all windows in-bounds (142)

_prog_cache: dict = {}


# --------------------------------------------------------------------------- #
# custom DVE ops (registered at import into concourse.dve_ops)                #
# --------------------------------------------------------------------------- #

def _register_ops():
    from concourse import dve_ops
    from concourse.dve_spec import (
        Spec, Src0, Src1, C0, C1, C2, Zero, lower, select, _has_src1,
    )
    from concourse.dve_uop import DveOpSpec

    if "ANT_ROUTE_BIN8" in dve_ops._SUB_OPCODE_FOR_NAME:
        ops = {op.name: op for op in dve_ops.OPS}
        return ops["ANT_ROUTE_BIN8"], ops["ANT_SEL_BIN8"]

    def reg(name, spec, subdim=False):
        row = dve_ops._CUSTOM_DVE_ROW_BASE + len(dve_ops.OPS)
        assert row < 0x20, "custom DVE opcode rows exhausted"
        dve_ops._SUB_OPCODE_FOR_NAME[name] = row
        shas = {}
        for ver in ("v3", "v4"):
            try:
                s = DveOpSpec(name=name, opcode=row,
                              uops=lower(spec, ver=ver),
                              rd1_en=_has_src1(spec))
                shas[ver] = s.sha(ver)
            except Exception:
                pass
        op = dve_ops.DveOp(name, spec, subdim=subdim, uops_sha=shas)
        dve_ops.OPS.append(op)
        dve_ops.CUSTOM_DVE_SPECS[name] = spec
        return op

    def _bc(v):
        return v if isinstance(v, float) else np.asarray(v).reshape(-1, 1)

    # s = x if (x >= lo) & (x < hi) else 0   (lo/hi per-partition scalars)
    route = reg("ANT_ROUTE_BIN8", Spec(
        body=select((Src0 >= C0) & (Src0 < C1), Src0, Zero),
        reference=lambda in0, in1, s0, s1, imm2: np.where(
            (in0 >= _bc(s0)) & (in0 < _bc(s1)), in0, 0.0).astype(np.float32),
    ))

    # masked = (x >= lo) & (x < lo + width) ? g + bias : 0
    #   in0 = g (PSUM), in1 = x (center voxel), s0 = lo, s1 = bias,
    #   imm2 = 1/8 bin width (compile-time literal)
    selb = reg("ANT_SEL_BIN8", Spec(
        body=select((Src1 >= C0) & (Src1 < (C0 + C2)), Src0 + C1, Zero),
        reference=lambda in0, in1, s0, s1, imm2: np.where(
            (in1 >= _bc(s0)) & (in1 < (_bc(s0) + imm2)),
            in0 + _bc(s1), 0.0).astype(np.float32),
    ))
    return route, selb


# --------------------------------------------------------------------------- #
# host-side constant construction                                             #
# --------------------------------------------------------------------------- #

def _corr_lo_pairs(drops):
    """Pairs of taps for the w_lo c-border correction DRs, grouped per side
    (both slots of a DR accumulate into the same pc region)."""
    pairs = []
    for side, lst in ((0, [t for t in (3, 4, 5) if t not in drops]),
                      (1, [t for t in (6, 7, 8) if t not in drops])):
        for k in range(0, len(lst), 2):
            pairs.append((side, lst[k], lst[k + 1] if k + 1 < len(lst)
                          else None))
    return pairs


def _band_lhsT(dw_kernel, co_matrix, dwi, dci):
    """lhsT[(q,hs),(p,hs')] = K[dh+1, dwi+1, dci+1, p] * co[p,q], dh=hs-hs',
    for hs' in [1,15), |dh| <= 1."""
    K = np.asarray(dw_kernel, np.float32)       # [3,3,3,1,8]
    co = np.asarray(co_matrix, np.float32)      # [8,8]
    lhsT = np.zeros((128, 128), np.float32)
    hsp = np.arange(1, 15)
    for q in range(NB):
        for p in range(NB):
            for dh in (-1, 0, 1):
                a = K[dh + 1, dwi + 1, dci + 1, 0, p] * co[p, q]
                lhsT[q * WIN + hsp + dh, p * WIN + hsp] = a
    return lhsT


def _make_consts(co_matrix, dw_kernel, dw_bias):
    E4 = ml_dtypes.float8_e4m3
    bands = [_band_lhsT(dw_kernel, co_matrix, dwi, dci)
             for (dwi, dci) in TAPS]                       # 9 x [128,128] f32
    hi = [b.astype(E4).astype(np.float32) for b in bands]  # w_hi
    lo = [b - h for b, h in zip(bands, hi)]                # w_lo (exact resid)

    # drop the w_lo slots of the NDROP least-significant taps so the
    # remaining slots pack into (9-NDROP)/2 DoubleRow matmuls (program
    # structure depends on this choice; _build_program reads _DROP_TAPS).
    # Significance proxy: per-hypothesis std of the dropped term,
    # sum_{dh,q} w_lo^2 E[s_q^2] with E[s_q^2] = E[x^2 1(bin q)], x~U[0,1).
    K5 = np.asarray(dw_kernel, np.float32)
    co5 = np.asarray(co_matrix, np.float32)
    m2 = np.array([((q + 1) ** 3 - q ** 3) / (3.0 * 8 ** 3) for q in range(8)])
    prox = []
    for (dwi, dci) in TAPS:
        v = 0.0
        for dh in range(3):
            wt = K5[dh, dwi + 1, dci + 1, 0, :][:, None] * co5
            wlo = wt - wt.astype(E4).astype(np.float32)
            v = v + (wlo ** 2 * m2[None, :]).sum(axis=1)
        prox.append(np.sqrt(v).max())
    drops = tuple(sorted(int(t) for t in np.argsort(prox)[:NDROP]))
    _DROP_TAPS[0] = drops
    lo_taps = [t for t in range(9) if t not in drops]
    npair = len(lo_taps) // 2

    # conv DoubleRow weights: [128, 9+npair, 2, 128]
    cw8 = np.zeros((128, 9 + npair, 2, 128), np.float32)
    for t in range(9):
        cw8[:, t, 0, :] = hi[t]        # slot0: w_hi x s_hi
        cw8[:, t, 1, :] = hi[t]        # slot1: w_hi x s_lo
    for j in range(npair):
        t0, t1 = lo_taps[2 * j], lo_taps[2 * j + 1]
        cw8[:, 9 + j, 0, :] = lo[t0]   # slot0: w_lo[t0] x s_hi
        cw8[:, 9 + j, 1, :] = lo[t1]

    # c-border correction weights (subtract the wrap-around contamination):
    # j=0..5: slot0 hits the s_hi plane, slot1 the s_lo plane, both -w_hi
    #         (cancels w_hi*cont_hi + w_hi*cont_lo for each dci!=0 tap).
    # j=6..9: -w_lo slots, both on the s_hi plane, taps paired two at a
    #         time per side (cancels w_lo*cont_hi).
    # w_lo contamination is only cancelled for taps whose main w_lo slot
    # exists (a dropped tap added no w_lo contribution to cancel).
    lo_pairs = _corr_lo_pairs(drops)
    cwc = np.zeros((128, 6 + len(lo_pairs), 2, 128), np.float32)
    for j, ti in enumerate(CTAPS):
        cwc[:, j, 0, :] = -hi[ti]
        cwc[:, j, 1, :] = -hi[ti]
    for i, (side, t0, t1) in enumerate(lo_pairs):
        cwc[:, 6 + i, 0, :] = -lo[t0]
        if t1 is not None:
            cwc[:, 6 + i, 1, :] = -lo[t1]

    # red_w[hw][p*16+hs, h] = 1 iff h == 14*hw + hs - 1, hs in [1,15)
    red_w = np.zeros((NWIN, 128, 128), np.float32)
    for hw in range(NWIN):
        for p in range(NB):
            for hs in range(1, 15):
                h = 14 * hw + hs - 1
                if 0 <= h < H:
                    red_w[hw, p * WIN + hs, h] = 1.0
    part = np.arange(128)
    bins_lo = ((part // WIN) / NB).astype(np.float32).reshape(128, 1)
    bias_p = np.asarray(dw_bias, np.float32)[part // WIN].reshape(128, 1)
    return {
        "cw8": cw8.astype(E4),
        "cwc": cwc.astype(E4),
        "red_w": red_w.astype(ml_dtypes.bfloat16),
        "bins_lo": bins_lo,
        "bias_p": bias_p,
    }


def _shard(x, core):
    """Per-core input: [PH, WH, C] with zero h-halo rows and w-halo cols."""
    b, wh = core // 2, core % 2
    xp = np.zeros((PH, WH, C), np.float32)
    lo, hi = wh * WS - 1, wh * WS + WS + 1
    slo, shi = max(0, lo), min(W, hi)
    xp[1:H + 1, slo - lo:shi - lo, :] = x[b, :, slo:shi, :]
    return xp


# --------------------------------------------------------------------------- #
# device program                                                              #
# --------------------------------------------------------------------------- #

def _build_program(reps=1):
    import concourse.mybir as mybir
    import concourse.tile as tile
    from concourse import bacc
    import bass_rust

    def raw_ap(base_ap, dims, offset):
        a = base_ap.copy()
        a.ap = bass_rust.VecI64Pair(dims)
        a.offset = offset
        return a

    def raw_free(base_ap, dims, extra_off):
        """Keep the partition dim of an SBUF slice, replace its free dims."""
        a = base_ap.copy()
        a.ap = bass_rust.VecI64Pair([list(base_ap.ap[0])] + dims)
        a.offset = base_ap.offset + extra_off
        return a

    ROUTE, SELB = _register_ops()
    f32 = mybir.dt.float32
    bf16 = mybir.dt.bfloat16
    e4 = mybir.dt.float8e4
    DR = mybir.MatmulPerfMode.DoubleRow

    drops = _DROP_TAPS[0]
    lo_taps = [t for t in range(9) if t not in drops]
    npair = len(lo_taps) // 2
    corr_lo = _corr_lo_pairs(drops)
    NCW = 9 + npair
    NCC = 6 + len(corr_lo)
    nc = bacc.Bacc("TRN2", target_bir_lowering=False, debug=False)
    x_d = nc.dram_tensor("x_s", [PH, WH, C], f32, kind="ExternalInput")
    cw_d = nc.dram_tensor("cw8", [128, NCW, 2, 128], e4, kind="ExternalInput")
    cc_d = nc.dram_tensor("cwc", [128, NCC, 2, 128], e4, kind="ExternalInput")
    rw_d = nc.dram_tensor("red_w", [NWIN, 128, 128], bf16,
                          kind="ExternalInput")
    lo_d = nc.dram_tensor("bins_lo", [128, 1], f32, kind="ExternalInput")
    bi_d = nc.dram_tensor("bias_p", [128, 1], f32, kind="ExternalInput")
    out_d = nc.dram_tensor("out_s", [H, WS, C], f32, kind="ExternalOutput")

    with tile.TileContext(nc) as tc:
        with (
            tc.tile_pool(name="const", bufs=1) as cpool,
            tc.tile_pool(name="xr", bufs=2) as xrpool,
            tc.tile_pool(name="s16", bufs=3) as s16pool,
            tc.tile_pool(name="sl8", bufs=2) as sl8pool,
            tc.tile_pool(name="mk", bufs=5) as mkpool,
            tc.tile_pool(name="ost", bufs=2) as ostpool,
            tc.tile_pool(name="ps", bufs=5, space="PSUM") as pspool,
            tc.tile_pool(name="ps2", bufs=1, space="PSUM") as ps2pool,
            tc.tile_pool(name="pc", bufs=1, space="PSUM") as pcpool,
        ):
            # startup critical chain: lo/hi gate the first ROUTE, cw8 gates the
            # first conv matmul; load those before the bulkier rw (only
            # needed once the first chunk's SELB completes).
            # lo/bi ride the gpsimd queue so the sync queue's head starts the
            # first xrm window immediately.
            lo = cpool.tile([128, 1], f32)
            nc.gpsimd.dma_start(lo[:], lo_d[:])
            bi = cpool.tile([128, 1], f32)
            nc.gpsimd.dma_start(bi[:], bi_d[:])
            hi = cpool.tile([128, 1], f32)
            nc.vector.tensor_scalar_add(hi[:], lo[:], 1.0 / NB)
            # big consts go on the Act queue: the sync queue must start the
            # first xrm windows immediately and the Pool queue must not delay
            # the first s_lo sub; Act is idle until the first s_hi convert.
            cw = cpool.tile([128, NCW, 2, 128], e4)
            nc.scalar.dma_start(cw[:], cw_d[:])
            cc = cpool.tile([128, NCC, 2, 128], e4)
            nc.scalar.dma_start(cc[:], cc_d[:])
            rw = cpool.tile([128, NWIN * 128], bf16)
            nc.scalar.dma_start(
                rw[:, :],
                raw_ap(rw_d[0], [[128, 128], [128 * 128, NWIN], [1, 128]], 0))

            # PE p-state warmup: ~16 throwaway matmuls on zeroed data keep
            # the PE busy through its 3us clock ramp while the first x
            # window streams in, so real convs start at full clock.  The
            # huge priority offset parks them below all real work in the
            # scheduler's ready heap.
            warm = cpool.tile([128, 512], bf16)
            nc.vector.memset(warm[:], 0.0)
            wps = pspool.tile([128, 512], f32, tag="ps", name="warm_ps")
            with tc.high_priority(offset=-(1 << 20)):
                for i in range(16):
                    nc.tensor.matmul(wps[:], warm[:, 0:128], warm[:],
                                     start=(i == 0), stop=(i == 15))

            for rep in range(reps):
              for st in range(NSTRIPES):
                  wb = st * WSTR
                  # ---- load x replicated into (q, hs) per window ----------- #
                  # padded row r = h+1: window hw needs h = 14*hw-1 .. 14*hw+14
                  # -> padded rows 14*hw .. 14*hw+15, all in-bounds.
                  xrm = xrpool.tile([128, NWIN, FSTR], f32, tag="xrm")

                  def xdma(hw, st=st, xrm=xrm, halves=False):
                      # halves=True: two half-window DMAs so the first route
                      # can start sooner (startup only).
                      parts = ((0, FSTR // 2), (FSTR // 2, FSTR)) if halves \
                          else ((0, FSTR),)
                      for f0, f1 in parts:
                          nc.sync.dma_start(
                              xrm[:, hw, f0:f1],
                              raw_ap(x_d[0:WIN, 0:WSTR_IN, :],
                                     [[0, 8], [PP, WIN], [1, f1 - f0]],
                                     14 * hw * PP + st * WSTR * C + f0))

                  def xwin(hw):
                      return xrm[:, hw, :]

                  # ---- route to bf16, split into fp8 hi/lo planes ---------- #
                  # window DMAs are emitted with prefetch depth 2 so the first
                  # route isn't stuck behind 10 queued DMA configs; window
                  # processing is interleaved with the conv chunks below so
                  # each engine's FIFO stays in pipeline order.
                  xdma(0, halves=(st == 0))
                  xdma(1, halves=(st == 0))
                  sl8 = sl8pool.tile([128, NWIN, 2, WBLK], e4, tag="sl8")
                  nc.gpsimd.memset(sl8[:, :, :, 0:PAD], 0.0)
                  nc.gpsimd.memset(sl8[:, :, :, WBLK - PAD:WBLK], 0.0)

                  def process_window(hw, st=st, sl8=sl8, halves=False):
                      if hw + 2 < NWIN:
                          xdma(hw + 2)
                      s16 = s16pool.tile([128, FSTR], bf16, tag="s16",
                                         name=f"s16_{st}_{hw}")
                      if halves:
                          # thirds: shorter DVE ops so a ready select op is
                          # blocked behind at most ~1/3 of a route.
                          parts = ((0, 288), (288, 576), (576, 864), (864, FSTR))
                      else:
                          parts = ((0, FSTR),)
                      for f0, f1 in parts:
                          nc.vector._custom_dve(
                              ROUTE, out=s16[:, f0:f1],
                              in0=xwin(hw)[:, f0:f1], s0=lo[:], s1=hi[:])
                          nc.scalar.copy(sl8[:, hw, 0, PAD + f0:PAD + f1],
                                         s16[:, f0:f1])
                          nc.gpsimd.tensor_sub(
                              sl8[:, hw, 1, PAD + f0:PAD + f1],
                              s16[:, f0:f1], sl8[:, hw, 0, PAD + f0:PAD + f1])

                  # two-window lookahead: the 3-engine route->s_hi->s_lo chain
                  # takes about one window's PE time, so it must run a full
                  # window ahead of the conv chunks.  The very first windows
                  # are processed in halves to shorten the cold-start chain.
                  process_window(0, halves=True)
                  process_window(1, halves=(st == 0))
                  sl8b = sl8[:, 0, 0, 0:512]   # AP base for raw free-dim APs

                  # ---- conv + select + stripe-accumulated reduce ----------- #
                  # two independent single-bank accumulators (one per wc
                  # half) with bufs=1: each is drained well before the next
                  # stripe's first reduce into it, so double-buffering them
                  # would waste PSUM banks better spent on ps depth.
                  p2h = (ps2pool.tile([128, 512], f32, tag="p2a",
                                      name=f"p2a_{st}"),
                         ps2pool.tile([128, 512], f32, tag="p2b",
                                      name=f"p2b_{st}"))
                  pend = []

                  def flush_tail(pend=pend, st=st, p2h=p2h):
                      if not pend:
                          return
                      ps, hw, wc, pcs = pend.pop(0)
                      # fold the border corrections into ps, then select.
                      # element order (w, side): ps col = 64*w + 63*side,
                      # pcs elem = wc*16 + 8*side + w.
                      ps_b = raw_free(ps[:, 0:8], [[C, 8], [C - 1, 2]], 0)
                      pc_b = raw_free(pcs[:, 0:8], [[1, 8], [8, 2]], wc * 16)
                      mk = mkpool.tile([128, 512], bf16, tag="mk",
                                       name=f"mk_{st}_{hw}_{wc}")
                      xcen = xwin(hw)[:, (wc * 8 + 1) * C:(wc * 8 + 9) * C]
                      # the fix->select->reduce chain gates PE; let it win
                      # ties against lookahead work in the scheduler heap.
                      with tc.high_priority(offset=150):
                          nc.vector.tensor_add(ps_b, ps_b, pc_b)
                          nc.vector._custom_dve(
                              SELB, out=mk[:], in0=ps[:],
                              in1=xcen, s0=lo[:], s1=bi[:], imm2=1.0 / NB)
                      nc.tensor.matmul(
                          p2h[wc][:, :],
                          rw[:, hw * 128:(hw + 1) * 128], mk[:],
                          start=(hw == 0), stop=(hw == NWIN - 1))

                  for hw in range(NWIN):
                      if hw + 2 < NWIN:
                          process_window(hw + 2)
                      blk = hw * WPAIR
                      # per-window correction accumulator [wc, side, 8] and
                      # its SBUF drain (one DVE copy per window, not chunk).
                      pcw = pcpool.tile([128, 2, 2, 8], f32, tag="pc",
                                        name=f"pc_{st}_{hw}")
                      pcs = mkpool.tile([128, 32], f32, tag="pcs",
                                        name=f"pcs_{st}_{hw}")
                      for wc in range(WSTR // 8):
                          w0 = wc * 8 + 1
                          ps = pspool.tile([128, 512], f32, tag="ps",
                                           name=f"ps_{st}_{hw}_{wc}")
                          nmm = 0
                          # 9 cross-plane pairs: w_hi*(s_hi, s_lo)
                          for t, (dwi, dci) in enumerate(TAPS):
                              F = blk + PAD + (w0 + dwi) * C + dci
                              rhs = raw_free(sl8b, [[WBLK, 2], [1, 512]], F)
                              nc.tensor.matmul(
                                  ps[:], cw[:, t, :, :], rhs,
                                  start=(nmm == 0), stop=False, perf_mode=DR)
                              nmm += 1
                          # same-plane pairs: w_lo * s_hi, taps two at a time
                          for j in range(npair):
                              t0, t1 = lo_taps[2 * j], lo_taps[2 * j + 1]
                              dw0, dc0 = TAPS[t0]
                              dw1, dc1 = TAPS[t1]
                              F0 = blk + PAD + (w0 + dw0) * C + dc0
                              delta = (dw1 - dw0) * C + (dc1 - dc0)
                              rhs = raw_free(sl8b, [[delta, 2], [1, 512]], F0)
                              nc.tensor.matmul(
                                  ps[:], cw[:, 9 + j, :, :], rhs,
                                  start=False, stop=(j == npair - 1),
                                  perf_mode=DR)
                              nmm += 1
                          # c-border corrections (free=8, strided rhs) into
                          # pcw[:, wc, side, :]; side 0 fixes out c=0 (dci=-1
                          # taps), side 1 fixes out c=63 (dci=+1 taps).

                          def fc_off(ti):
                              dwi, dci = TAPS[ti]
                              if dci == -1:
                                  return blk + PAD + (w0 + dwi - 1) * C + (C - 1)
                              return blk + PAD + (w0 + dwi + 1) * C

                          ncorr = 6 + len(corr_lo)
                          for j, ti in enumerate(CTAPS):
                              side = 0 if TAPS[ti][1] == -1 else 1
                              rhs = raw_free(sl8b, [[WBLK, 2], [C, 8]],
                                             fc_off(ti))
                              nc.tensor.matmul(
                                  pcw[:, wc, side, :], cc[:, j, :, :], rhs,
                                  start=(j == 0), stop=False, perf_mode=DR)
                          # -w_lo slots (hi plane, taps paired per side)
                          for i, (side, t0, t1) in enumerate(corr_lo):
                              F0 = fc_off(t0)
                              if t1 is not None:
                                  delta = fc_off(t1) - F0
                              else:
                                  delta = C if side == 0 else -C
                              rhs = raw_free(sl8b, [[delta, 2], [C, 8]], F0)
                              nc.tensor.matmul(
                                  pcw[:, wc, side, :], cc[:, 6 + i, :, :], rhs,
                                  start=False, stop=(6 + i == ncorr - 1),
                                  perf_mode=DR)
                          pend.append((ps, hw, wc, pcs))
                          if wc == 1:
                              # drain both chunks' corrections to SBUF (the
                              # DVE add may read only one PSUM operand).
                              with tc.high_priority(offset=150):
                                  nc.scalar.copy(pcs[:, :], pcw[:, :, :, :])
                          if len(pend) > 2:
                              flush_tail()
                  while pend:
                      flush_tail()

                  # ---- drain stripe + store (two halves so the wc=0 half
                  # overlaps the wc=1 tail) --------------------------------- #
                  ost = ostpool.tile([128, 1024], f32, tag="ost",
                                     name=f"ost_{st}")
                  for wc in range(2):
                      nc.scalar.copy(ost[:, wc * 512:(wc + 1) * 512],
                                     p2h[wc][:, :])
                      # alternate output DMA queues so the two half-stores
                      # stream on parallel DMA engines (shortens the tail).
                      eng = nc.sync if wc == 0 else nc.scalar
                      eng.dma_start(
                          raw_ap(out_d[0:H, 0:WSTR, :],
                                 [[WS * C, 128], [1, 512]],
                                 wb * C + wc * 512),
                          ost[:, wc * 512:(wc + 1) * 512])

    nc.compile()
    return nc


def _get_program(reps=1):
    key = (reps, _DROP_TAPS[0])
    if key not in _prog_cache:
        _prog_cache[key] = _build_program(reps)
    return _prog_cache[key]


# --------------------------------------------------------------------------- #
# entry point                                                                 #
# --------------------------------------------------------------------------- #

def kernel(x, co_matrix, dw_kernel, dw_bias):
    from concourse.bass_utils import run_bass_kernel_spmd

    x = np.asarray(x, np.float32)
    consts = _make_consts(co_matrix, dw_kernel, dw_bias)
    nc = _get_program()

    in_maps = []
    for core in range(NCORES):
        m = {"x_s": _shard(x, core)}
        m.update(consts)
        in_maps.append(m)

    res = run_bass_kernel_spmd(nc, in_maps, core_ids=list(range(NCORES)))
    out = np.zeros((B, H, W, C), np.float32)
    for core in range(NCORES):
        b, wh = core // 2, core % 2
        out[b, :, wh * WS:(wh + 1) * WS, :] = res.results[core]["out_s"]
    return out
